# revision 6
# baseline (speedup 1.0000x reference)
"""Causal attention (B=8, S=2048, D=1024, fp32) on 8 TRN2 NeuronCores.

Sharding: batch-parallel, one batch element per core (SPMD, no collectives).

Per-core algorithm (S^T layout):
  - Q, K are loaded with an fp32->bf16 cast during DMA, then transposed on
    TensorE (128x128 tiles vs a bf16 identity) into [d, s] layouts QT/KT.
  - Scores are computed transposed: S^T[k, q] = sum_d KT[d,k] * QT[d,q],
    accumulated over 8 d-subtiles in PSUM, 2 k-tiles x 256 q per PSUM bank.
  - Causal mask: additive -1e10 on the diagonal pair only (precomputed
    [128, 2, 256] mask via affine_select); k-tiles above the diagonal are
    skipped entirely.
  - exp((dots+mask)/sqrt(D)) on ScalarE (no max subtraction: |dots| <= ~1.1e3
    so logits <= ~35, exp fits fp32 comfortably), output cast to bf16 = P^T.
  - PV: O[q, d] += P^T.T @ V with V in native [k, d] layout; row sums via an
    extra N=1 matmul against a ones vector; final normalization is a DVE
    multiply by the reciprocal row sum (numerator/denominator both built from
    the same bf16 P^T, so rounding cancels to first order).
"""

import numpy as np

import concourse.bass as bass
import concourse.mybir as mybir
import concourse.tile as tile
from concourse import bacc
from concourse.masks import make_identity

P = 128
MASKVAL = -1e10  # matches reference INF (subtracted pre-scale)


def build_attention_nc(S=2048, D=1024):
    f32, bf16 = mybir.dt.float32, mybir.dt.bfloat16
    nc = bacc.Bacc(None, target_bir_lowering=False)

    q_d = nc.dram_tensor("query", [S, D], f32, kind="ExternalInput")
    k_d = nc.dram_tensor("key", [S, D], f32, kind="ExternalInput")
    v_d = nc.dram_tensor("value", [S, D], f32, kind="ExternalInput")
    o_d = nc.dram_tensor("out", [S, D], f32, kind="ExternalOutput")

    NT = S // P            # number of 128-row seq tiles
    ND = D // P            # number of 128-wide d subtiles
    QGT = 2                # q-tiles per group
    QG = QGT * P           # q-group width (256)
    NG = S // QG           # number of q groups
    DH = min(D, 512)       # PV free-dim chunk (one PSUM bank)
    NDH = D // DH
    TCH = 4 if ND % 4 == 0 else (2 if ND % 2 == 0 else 1)  # transpose chunk
    scale = 1.0 / float(np.sqrt(D))

    qv = q_d.rearrange("(n p) d -> p n d", p=P)
    kv = k_d.rearrange("(n p) d -> p n d", p=P)
    vv = v_d.rearrange("(n p) d -> p n d", p=P)
    ov = o_d.rearrange("(n p) d -> p n d", p=P)

    with tile.TileContext(nc) as tc:
        with (
            tc.tile_pool(name="const", bufs=1) as constp,
            tc.tile_pool(name="slab", bufs=1) as slab,
            tc.tile_pool(name="stage", bufs=8) as stagep,
            tc.tile_pool(name="pt", bufs=3) as ptp,
            tc.tile_pool(name="small", bufs=2) as smallp,
            tc.tile_pool(name="ost", bufs=2) as ostp,
            tc.tile_pool(name="ps", bufs=1, space="PSUM") as psp,
        ):
            ident = constp.tile([P, P], bf16)
            make_identity(nc, ident[:])
            ones = constp.tile([P, 1], bf16)
            nc.vector.memset(ones[:], 1.0)

            # Additive causal mask for the diagonal k-tile pair, S^T layout:
            # maskt[kk, half, qq] = 0 if (128*half + kk) <= qq else MASKVAL
            maskt = constp.tile([P, 2, QG], f32)
            for half in range(2):
                m = maskt[:, half, :]
                nc.gpsimd.memset(m, 0.0)
                nc.gpsimd.affine_select(
                    out=m,
                    in_=m,
                    compare_op=mybir.AluOpType.is_ge,
                    fill=MASKVAL,
                    base=-(P * half),
                    pattern=[[1, QG]],
                    channel_multiplier=-1,
                )

            QT = slab.tile([P, ND, S], bf16)   # [d%128, d//128, q]
            KT = slab.tile([P, ND, S], bf16)   # [d%128, d//128, k]
            V = slab.tile([P, NT, D], bf16)    # [k%128, k//128, d]

            def emit_loads(g):
                """Issue the cast-DMAs for group g's new Q/K/V tiles."""
                stages = {}
                for t in range(QGT * g, QGT * (g + 1)):
                    for nm, srcv in (("q", qv), ("k", kv)):
                        stg = stagep.tile([P, D], bf16, tag="stage", name=f"stg_{nm}{t}")
                        nc.gpsimd.dma_start(stg[:], srcv[:, t, :])  # fp32->bf16
                        stages[(nm, t)] = stg
                    nc.gpsimd.dma_start(V[:, t, :], vv[:, t, :])  # fp32->bf16
                return stages

            def emit_transposes(g, stages):
                # Q tiles first: group g's QK matmuls need QT immediately,
                # but the new KT tiles only at the diagonal (last) pair.
                for nm, dst in (("q", QT), ("k", KT)):
                    for t in range(QGT * g, QGT * (g + 1)):
                        stg = stages[(nm, t)]
                        for c in range(ND // TCH):
                            pst = psp.tile([P, TCH, P], bf16, tag="st", bufs=3)
                            for j in range(TCH):
                                ds = c * TCH + j
                                nc.tensor.transpose(
                                    pst[:, j, :],
                                    stg[:, ds * P : (ds + 1) * P],
                                    ident[:],
                                )
                            dslc = dst[:, c * TCH : (c + 1) * TCH, t * P : (t + 1) * P]
                            if nm == "q":
                                nc.vector.tensor_copy(dslc, pst[:])
                            else:
                                nc.scalar.copy(dslc, pst[:])

            pending = emit_loads(0)
            for g in range(NG):
                # Prefetch next group's DMA loads before anything else so
                # they land while this group's pair loop runs.
                nxt = emit_loads(g + 1) if g + 1 < NG else None
                emit_transposes(g, pending)
                pending = nxt

                # ---- score + softmax + PV over k-tile pairs ----
                opv = [
                    psp.tile([P, D], f32, tag=f"pv{j}", bufs=1, name=f"opv{j}")
                    for j in range(QGT)
                ]
                rsps = psp.tile([P, QGT], f32, tag="rs", bufs=1)
                for p in range(g + 1):
                    diag = p == g
                    stps = psp.tile([P, 2, QG], f32, tag="st", bufs=3)
                    for kk in range(2):
                        ki = 2 * p + kk
                        # Diagonal pair, second k-tile: q < 128 (rel) is fully
                        # masked, so only compute the upper q half (N=128).
                        qlo = P if (diag and kk == 1) else 0
                        for ds in range(ND):
                            nc.tensor.matmul(
                                stps[:, kk, qlo:],
                                lhsT=KT[:, ds, ki * P : (ki + 1) * P],
                                rhs=QT[:, ds, g * QG + qlo : (g + 1) * QG],
                                start=(ds == 0),
                                stop=(ds == ND - 1),
                            )
                    if diag:
                        nc.vector.tensor_add(
                            stps[:, 0, :], stps[:, 0, :], maskt[:, 0, :]
                        )
                        # The uncomputed quarter never got written: set it to
                        # the mask value directly so exp maps it to 0.
                        nc.vector.memset(stps[:, 1, :P], MASKVAL)
                        nc.vector.tensor_add(
                            stps[:, 1, P:], stps[:, 1, P:], maskt[:, 1, P:]
                        )
                    ptt = ptp.tile([P, 2, QG], bf16, tag="pt")
                    nc.scalar.activation(
                        ptt[:], stps[:], mybir.ActivationFunctionType.Exp,
                        scale=scale,
                    )
                    for kk in range(2):
                        ki = 2 * p + kk
                        first = (p == 0) and (kk == 0)
                        for j in range(QGT):
                            if diag and kk == 1 and j == 0:
                                continue  # fully masked block
                            # last matmul touching opv[j]'s accumulation:
                            last_j = diag and (kk == 1 or (kk == 0 and j == 0))
                            lh = ptt[:, kk, j * P : (j + 1) * P]
                            for dh in range(NDH):
                                nc.tensor.matmul(
                                    opv[j][:, dh * DH : (dh + 1) * DH],
                                    lhsT=lh,
                                    rhs=V[:, ki, dh * DH : (dh + 1) * DH],
                                    start=first,
                                    stop=last_j,
                                )
                            # rsps is one PSUM bank = one zero region: start
                            # exactly once (marks whole bank pending-zero, so
                            # each column's first write lands as overwrite).
                            nc.tensor.matmul(
                                rsps[:, j : j + 1],
                                lhsT=lh,
                                rhs=ones[:],
                                start=(first and j == 0),
                                stop=(diag and kk == 1 and j == QGT - 1),
                            )

                # ---- normalize + store (split across DVE and ACT) ----
                rec = smallp.tile([P, QGT], f32, tag="rec")
                nc.vector.reciprocal(rec[:], rsps[:])
                for j in range(QGT):
                    ost = ostp.tile([P, D], f32, tag="ost")
                    nc.vector.tensor_scalar_mul(
                        ost[:, :DH], opv[j][:, :DH], scalar1=rec[:, j : j + 1]
                    )
                    nc.sync.dma_start(ov[:, g * QGT + j, :DH], ost[:, :DH])
                    if D > DH:
                        nc.scalar.mul(
                            ost[:, DH:], opv[j][:, DH:], mul=rec[:, j : j + 1]
                        )
                        nc.sync.dma_start(ov[:, g * QGT + j, DH:], ost[:, DH:])

    nc.compile()
    return nc


_NC_CACHE = {}


def _get_nc(S, D):
    if (S, D) not in _NC_CACHE:
        _NC_CACHE[(S, D)] = build_attention_nc(S, D)
    return _NC_CACHE[(S, D)]


def kernel(query, key, value):
    from concourse.bass_utils import run_bass_kernel_spmd

    query = np.asarray(query, dtype=np.float32)
    key = np.asarray(key, dtype=np.float32)
    value = np.asarray(value, dtype=np.float32)
    B, S, D = query.shape
    nc = _get_nc(S, D)
    in_maps = [
        {
            "query": np.ascontiguousarray(query[i]),
            "key": np.ascontiguousarray(key[i]),
            "value": np.ascontiguousarray(value[i]),
        }
        for i in range(B)
    ]
    res = run_bass_kernel_spmd(nc, in_maps, core_ids=list(range(B)))
    out = np.stack([r["out"] for r in res.results], axis=0)
    return out.astype(np.float32)


# revision 7
# speedup vs baseline: 1.2040x; 1.2040x over previous
"""Causal attention (B=8, S=2048, D=1024, fp32) on 8 TRN2 NeuronCores.

Sharding: batch-parallel, one batch element per core (SPMD, no collectives).

Per-core algorithm (S^T layout):
  - Q, K are loaded with an fp32->bf16 cast during DMA, then transposed on
    TensorE (128x128 tiles vs a bf16 identity) into [d, s] layouts QT/KT.
  - Scores are computed transposed: S^T[k, q] = sum_d KT[d,k] * QT[d,q],
    accumulated over 8 d-subtiles in PSUM, 2 k-tiles x 256 q per PSUM bank.
  - Causal mask: additive -1e10 on the diagonal pair only (precomputed
    [128, 2, 256] mask via affine_select); k-tiles above the diagonal are
    skipped entirely.
  - exp((dots+mask)/sqrt(D)) on ScalarE (no max subtraction: |dots| <= ~1.1e3
    so logits <= ~35, exp fits fp32 comfortably), output cast to bf16 = P^T.
  - PV: O[q, d] += P^T.T @ V with V in native [k, d] layout; row sums via an
    extra N=1 matmul against a ones vector; final normalization is a DVE
    multiply by the reciprocal row sum (numerator/denominator both built from
    the same bf16 P^T, so rounding cancels to first order).
"""

import numpy as np

import concourse.bass as bass
import concourse.mybir as mybir
import concourse.tile as tile
from concourse import bacc
from concourse.masks import make_identity

P = 128
MASKVAL = -1e10  # matches reference INF (subtracted pre-scale)


def build_attention_nc(S=2048, D=1024):
    f32, bf16 = mybir.dt.float32, mybir.dt.bfloat16
    nc = bacc.Bacc(None, target_bir_lowering=False)

    q_d = nc.dram_tensor("query", [S, D], f32, kind="ExternalInput")
    k_d = nc.dram_tensor("key", [S, D], f32, kind="ExternalInput")
    v_d = nc.dram_tensor("value", [S, D], f32, kind="ExternalInput")
    o_d = nc.dram_tensor("out", [S, D], f32, kind="ExternalOutput")

    NT = S // P            # number of 128-row seq tiles
    ND = D // P            # number of 128-wide d subtiles
    QGT = 2                # q-tiles per group
    QG = QGT * P           # q-group width (256)
    NG = S // QG           # number of q groups
    DH = min(D, 512)       # PV free-dim chunk (one PSUM bank)
    NDH = D // DH
    TCH = 4 if ND % 4 == 0 else (2 if ND % 2 == 0 else 1)  # transpose chunk
    scale = 1.0 / float(np.sqrt(D))

    qv = q_d.rearrange("(n p) d -> p n d", p=P)
    kv = k_d.rearrange("(n p) d -> p n d", p=P)
    vv = v_d.rearrange("(n p) d -> p n d", p=P)
    ov = o_d.rearrange("(n p) d -> p n d", p=P)

    with tile.TileContext(nc) as tc:
        with (
            tc.tile_pool(name="const", bufs=1) as constp,
            tc.tile_pool(name="slab", bufs=1) as slab,
            tc.tile_pool(name="stage", bufs=8) as stagep,
            tc.tile_pool(name="pt", bufs=3) as ptp,
            tc.tile_pool(name="small", bufs=2) as smallp,
            tc.tile_pool(name="ost", bufs=2) as ostp,
            tc.tile_pool(name="ps", bufs=1, space="PSUM") as psp,
        ):
            ident = constp.tile([P, P], bf16)
            make_identity(nc, ident[:])
            ones = constp.tile([P, 1], bf16)
            nc.vector.memset(ones[:], 1.0)

            # Additive causal mask for the diagonal k-tile pair, S^T layout:
            # maskt[kk, half, qq] = 0 if (128*half + kk) <= qq else MASKVAL
            maskt = constp.tile([P, 2, QG], f32)
            for half in range(2):
                m = maskt[:, half, :]
                nc.gpsimd.memset(m, 0.0)
                nc.gpsimd.affine_select(
                    out=m,
                    in_=m,
                    compare_op=mybir.AluOpType.is_ge,
                    fill=MASKVAL,
                    base=-(P * half),
                    pattern=[[1, QG]],
                    channel_multiplier=-1,
                )

            QT = slab.tile([P, ND, S], bf16)   # [d%128, d//128, q]
            KT = slab.tile([P, ND, S], bf16)   # [d%128, d//128, k]
            V = slab.tile([P, NT, D], bf16)    # [k%128, k//128, d]

            def emit_loads(g):
                """Issue the cast-DMAs for group g's new Q/K/V tiles."""
                stages = {}
                for t in range(QGT * g, QGT * (g + 1)):
                    for nm, srcv in (("q", qv), ("k", kv)):
                        stg = stagep.tile([P, D], bf16, tag="stage", name=f"stg_{nm}{t}")
                        nc.gpsimd.dma_start(stg[:], srcv[:, t, :])  # fp32->bf16
                        stages[(nm, t)] = stg
                    nc.gpsimd.dma_start(V[:, t, :], vv[:, t, :])  # fp32->bf16
                return stages

            def emit_transposes(g, stages):
                # Q tiles first: group g's QK matmuls need QT immediately,
                # but the new KT tiles only at the diagonal (last) pair.
                for nm, dst in (("q", QT), ("k", KT)):
                    for t in range(QGT * g, QGT * (g + 1)):
                        stg = stages[(nm, t)]
                        for c in range(ND // TCH):
                            pst = psp.tile([P, TCH, P], bf16, tag="st", bufs=3)
                            for j in range(TCH):
                                ds = c * TCH + j
                                nc.tensor.transpose(
                                    pst[:, j, :],
                                    stg[:, ds * P : (ds + 1) * P],
                                    ident[:],
                                )
                            dslc = dst[:, c * TCH : (c + 1) * TCH, t * P : (t + 1) * P]
                            if nm == "q":
                                nc.vector.tensor_copy(dslc, pst[:])
                            else:
                                nc.scalar.copy(dslc, pst[:])

            pending = emit_loads(0)
            for g in range(NG):
                # Prefetch next group's DMA loads before anything else so
                # they land while this group's pair loop runs.
                nxt = emit_loads(g + 1) if g + 1 < NG else None
                emit_transposes(g, pending)
                pending = nxt

                # ---- score + softmax + PV over k-tile pairs ----
                opv = [
                    psp.tile([P, D], f32, tag=f"pv{j}", bufs=1, name=f"opv{j}")
                    for j in range(QGT)
                ]
                rsps = psp.tile([P, QGT], f32, tag="rs", bufs=1)
                for p in range(g + 1):
                    diag = p == g
                    stps = psp.tile([P, 2, QG], f32, tag="st", bufs=3)
                    for kk in range(2):
                        ki = 2 * p + kk
                        # Diagonal pair, second k-tile: q < 128 (rel) is fully
                        # masked, so only compute the upper q half (N=128).
                        qlo = P if (diag and kk == 1) else 0
                        for ds in range(ND):
                            nc.tensor.matmul(
                                stps[:, kk, qlo:],
                                lhsT=KT[:, ds, ki * P : (ki + 1) * P],
                                rhs=QT[:, ds, g * QG + qlo : (g + 1) * QG],
                                start=(ds == 0),
                                stop=(ds == ND - 1),
                            )
                    if diag:
                        nc.vector.tensor_add(
                            stps[:, 0, :], stps[:, 0, :], maskt[:, 0, :]
                        )
                        # The uncomputed quarter never got written: set it to
                        # the mask value directly so exp maps it to 0.
                        nc.vector.memset(stps[:, 1, :P], MASKVAL)
                        nc.vector.tensor_add(
                            stps[:, 1, P:], stps[:, 1, P:], maskt[:, 1, P:]
                        )
                    ptt = ptp.tile([P, 2, QG], bf16, tag="pt")
                    nc.scalar.activation(
                        ptt[:], stps[:], mybir.ActivationFunctionType.Exp,
                        scale=scale,
                    )
                    for kk in range(2):
                        ki = 2 * p + kk
                        first = (p == 0) and (kk == 0)
                        for j in range(QGT):
                            if diag and kk == 1 and j == 0:
                                continue  # fully masked block
                            # last matmul touching opv[j]'s accumulation:
                            last_j = diag and (kk == 1 or (kk == 0 and j == 0))
                            lh = ptt[:, kk, j * P : (j + 1) * P]
                            for dh in range(NDH):
                                nc.tensor.matmul(
                                    opv[j][:, dh * DH : (dh + 1) * DH],
                                    lhsT=lh,
                                    rhs=V[:, ki, dh * DH : (dh + 1) * DH],
                                    start=first,
                                    stop=last_j,
                                )
                            # rsps is one PSUM bank = one zero region: start
                            # exactly once (marks whole bank pending-zero, so
                            # each column's first write lands as overwrite).
                            nc.tensor.matmul(
                                rsps[:, j : j + 1],
                                lhsT=lh,
                                rhs=ones[:],
                                start=(first and j == 0),
                                stop=(diag and kk == 1 and j == QGT - 1),
                            )

                # ---- normalize + store ----
                rec = smallp.tile([P, QGT], f32, tag="rec")
                nc.vector.reciprocal(rec[:], rsps[:])
                for j in range(QGT):
                    ost = ostp.tile([P, D], f32, tag="ost")
                    nc.vector.tensor_scalar_mul(
                        ost[:], opv[j][:], scalar1=rec[:, j : j + 1]
                    )
                    nc.sync.dma_start(ov[:, g * QGT + j, :], ost[:])

    nc.compile()
    return nc


_NC_CACHE = {}


def _get_nc(S, D):
    if (S, D) not in _NC_CACHE:
        _NC_CACHE[(S, D)] = build_attention_nc(S, D)
    return _NC_CACHE[(S, D)]


def kernel(query, key, value):
    from concourse.bass_utils import run_bass_kernel_spmd

    query = np.asarray(query, dtype=np.float32)
    key = np.asarray(key, dtype=np.float32)
    value = np.asarray(value, dtype=np.float32)
    B, S, D = query.shape
    nc = _get_nc(S, D)
    in_maps = [
        {
            "query": np.ascontiguousarray(query[i]),
            "key": np.ascontiguousarray(key[i]),
            "value": np.ascontiguousarray(value[i]),
        }
        for i in range(B)
    ]
    res = run_bass_kernel_spmd(nc, in_maps, core_ids=list(range(B)))
    out = np.stack([r["out"] for r in res.results], axis=0)
    return out.astype(np.float32)
